# revision 5
# baseline (speedup 1.0000x reference)
"""Bahdanau attention TRN2 Bass kernel.

Full inputs in, full outputs out; batch-sharded across 8 NeuronCores
(8 batches per core), computed entirely on-device per core (no collectives).

Per-core per-batch dataflow (all fp32 bytes; PE ops run in float32r mode,
~12-bit-mantissa rounding, max rel err ~2.4e-4):
  1. DMA enc[b] (2048x1024) into SBUF natural chunks [128s, 4, 1024e].
  2. PE-transpose enc blocks -> encT tiles [128e, 512s]  (contraction over E
     needs E on partitions; fp32 DMA transpose is unsupported on TRN2).
  3. Main matmul: enc_featT[a,s] = W_encT.T @ encT, accumulated over 8 e-tiles.
  4. ACT tanh with per-partition bias dec_featT[:,b] straight out of PSUM.
  5. Scores: W_energyT.T @ energyT (M=1 matvec on PE), + mask*FLT_MIN via the
     PSUM->SBUF copy as a DVE tensor_tensor add.
  6. Softmax without max-subtraction (scores are O(1); masked lanes are
     -3.4e38 -> exp==0): ACT Exp with fused accum_out sum, DVE reciprocal,
     DVE tensor_scalar normalize.
  7. Context: attnT (PE-transposed attention row) as stationary, natural enc
     chunks as moving operand, accumulated over 16 s-tiles.
Batches are software-pipelined: batch b's attnT/context work is emitted after
batch b+1's first section of transposes/matmuls so the PE never stalls on the
softmax latency.
"""
import sys

sys.path.insert(0, "/opt/trn_rl_repo")

import numpy as np
from contextlib import ExitStack

import concourse.bass as bass
import concourse.tile as tile
from concourse import bacc, mybir
from concourse.bass_utils import run_bass_kernel_spmd
from concourse.masks import make_identity

dt = mybir.dt
F32 = dt.float32
F32R = dt.float32r
U8 = dt.uint8
Alu = mybir.AluOpType
Act = mybir.ActivationFunctionType

B, S, E, A = 64, 2048, 1024, 512
NCORES = 8
BL = B // NCORES            # 8 batches per core
NST = S // 128              # 16 s-tiles per batch
NSC = 4                     # s-chunks per batch (512 s values each)
NET = E // 128              # 8 e-tiles
NAT = A // 128              # 4 a-tiles
FMIN = float(np.finfo(np.float32).min)


def build_program():
    nc = bacc.Bacc(None, target_bir_lowering=False)

    enc_in = nc.declare_dram_parameter("enc", [BL, S, E], F32R, isOutput=False)
    dh_in = nc.declare_dram_parameter("dh", [BL, E], F32R, isOutput=False)
    mask_in = nc.declare_dram_parameter("mask", [BL, S], U8, isOutput=False)
    wenc_in = nc.declare_dram_parameter("w_enc", [A, E], F32R, isOutput=False)
    wdec_in = nc.declare_dram_parameter("w_dec", [A, E], F32R, isOutput=False)
    wene_in = nc.declare_dram_parameter("w_energy", [1, A], F32R, isOutput=False)
    ctx_out = nc.declare_dram_parameter("ctx", [BL, E], F32, isOutput=True)
    attn_out = nc.declare_dram_parameter("attn", [BL, S], F32R, isOutput=True)

    with tile.TileContext(nc) as tc, ExitStack() as ctx:
        persist = ctx.enter_context(tc.tile_pool(name="persist", bufs=1))
        p_t = ctx.enter_context(tc.tile_pool(name="p_t", bufs=2, space="PSUM"))
        p_m = ctx.enter_context(tc.tile_pool(name="p_m", bufs=2, space="PSUM"))
        p_s = ctx.enter_context(tc.tile_pool(name="p_s", bufs=1, space="PSUM"))
        p_a = ctx.enter_context(tc.tile_pool(name="p_a", bufs=1, space="PSUM"))
        p_c = ctx.enter_context(tc.tile_pool(name="p_c", bufs=2, space="PSUM"))

        # ---- startup: weights ----
        w_encT = []   # 8 tiles [128e, 512a] f32r
        dec_featT = persist.tile([128, NAT * BL], F32, tag="dfT")  # col = at*8+b
        w_eneT = persist.tile([128, 2 * NAT], F32R, tag="weT")  # cols 2*at (odd cols zero)
        ident = persist.tile([128, 128], F32R, tag="ident")

        with tc.tile_pool(name="startup", bufs=1) as st:
            ident32 = st.tile([128, 128], F32, tag="ident32")
            make_identity(nc, ident32[:])
            nc.vector.tensor_copy(ident[:], ident32[:])
            wenc_nat = []
            wdec_nat = []
            for at in range(NAT):
                wn = st.tile([128, E], F32R, tag=f"wen{at}")
                nc.sync.dma_start(wn[:], wenc_in[at * 128:(at + 1) * 128, :])
                wenc_nat.append(wn)
                wd = st.tile([128, E], F32R, tag=f"wdn{at}")
                nc.sync.dma_start(wd[:], wdec_in[at * 128:(at + 1) * 128, :])
                wdec_nat.append(wd)
            dh_sb = st.tile([BL, E], F32R, tag="dh")
            nc.sync.dma_start(dh_sb[:], dh_in[:])
            wene_sb = st.tile([1, A], F32R, tag="wene")
            nc.sync.dma_start(wene_sb[:], wene_in[:])

            # W_encT tiles (persist pool)
            for et in range(NET):
                ps = p_t.tile([128, A], F32R, tag="t")
                for at in range(NAT):
                    nc.tensor.transpose(
                        ps[:, at * 128:(at + 1) * 128],
                        wenc_nat[at][:, et * 128:(et + 1) * 128],
                        ident[:],
                    )
                wt = persist.tile([128, A], F32R, tag=f"wencT{et}")
                if et % 2 == 0:
                    nc.vector.tensor_copy(wt[:], ps[:])
                else:
                    nc.scalar.copy(wt[:], ps[:])
                w_encT.append(wt)

            # W_decT + dhT then dec_featT = W_dec @ dh.T  -> [a, b]
            w_decT = []
            dhT = []
            for et in range(NET):
                ps = p_t.tile([128, A], F32R, tag="t")
                for at in range(NAT):
                    nc.tensor.transpose(
                        ps[:, at * 128:(at + 1) * 128],
                        wdec_nat[at][:, et * 128:(et + 1) * 128],
                        ident[:],
                    )
                wt = st.tile([128, A], F32R, tag=f"wdecT{et}")
                nc.vector.tensor_copy(wt[:], ps[:])
                w_decT.append(wt)
                psd = p_t.tile([128, BL], F32R, tag="t")
                nc.tensor.transpose(psd[:, :], dh_sb[:, et * 128:(et + 1) * 128],
                                    ident[0:BL, 0:BL])
                dt_t = st.tile([128, BL], F32R, tag=f"dhT{et}")
                nc.vector.tensor_copy(dt_t[:], psd[:])
                dhT.append(dt_t)
            for at in range(NAT):
                psm = p_m.tile([128, BL], F32, tag="m")
                for et in range(NET):
                    nc.tensor.matmul(
                        psm[:], w_decT[et][:, at * 128:(at + 1) * 128], dhT[et][:],
                        start=(et == 0), stop=(et == NET - 1),
                    )
                nc.scalar.copy(dec_featT[:, at * BL:(at + 1) * BL], psm[:])

            # W_energyT: fp32r moving operand needs even innermost count, so
            # transpose against ident[0:1, 0:2] = [1, 0] -> real col + zero col
            pse = p_t.tile([128, 2 * NAT], F32R, tag="t")
            for at in range(NAT):
                nc.tensor.transpose(
                    pse[:, 2 * at:2 * at + 2],
                    wene_sb[0:1, at * 128:(at + 1) * 128],
                    ident[0:1, 0:2],
                )
            nc.vector.tensor_copy(w_eneT[:], pse[:])

        # ---- steady-state pools ----
        enc_pool = ctx.enter_context(tc.tile_pool(name="encp", bufs=6))
        encT_pool = ctx.enter_context(tc.tile_pool(name="encTp", bufs=10))
        egyT_pool = ctx.enter_context(tc.tile_pool(name="egyTp", bufs=6))
        rows = ctx.enter_context(tc.tile_pool(name="rows", bufs=2))
        small = ctx.enter_context(tc.tile_pool(name="small", bufs=2))

        secs = [(b, sc) for b in range(BL) for sc in range(NSC)]
        enc_chunks = {}      # (b, sc) -> tile [128, 4, 1024]
        mask_rows = {}       # b -> [1, S] f32 tile
        scores_rows = {}     # b -> [1, S] f32
        attn_rows = {}       # b -> [1, S] f32r

        def emit_chunk_dma(b, sc):
            t = enc_pool.tile([128, NSC, E], F32R, tag="enc")
            src = enc_in[b, sc * 512:(sc + 1) * 512, :].rearrange(
                "(t p) e -> p t e", p=128)
            nc.sync.dma_start(t[:], src)
            enc_chunks[(b, sc)] = t

        def emit_mask_row(b):
            mu = small.tile([1, S], U8, tag="mask_u8")
            nc.sync.dma_start(mu[:], mask_in[b:b + 1, :])
            mf = rows.tile([1, S], F32, tag="mask_f")
            nc.vector.tensor_scalar(mf[:], mu[:], FMIN, None, op0=Alu.mult)
            mask_rows[b] = mf

        def emit_attnT_ctx(b):
            # attention row -> attnT [128s, 16(si)] via 16 PE transposes
            arow = attn_rows[b]
            psa = p_a.tile([128, 2 * NST], F32R, tag="a")
            for si in range(NST):
                nc.tensor.transpose(
                    psa[:, 2 * si:2 * si + 2],
                    arow[0:1, si * 128:(si + 1) * 128],
                    ident[0:1, 0:2],
                )
            attnT = small.tile([128, 2 * NST], F32R, tag="attnT")
            nc.vector.tensor_copy(attnT[:], psa[:])
            # context: ctx[1, e] = sum_s attn[s] * enc[s, e]
            crow = rows.tile([1, E], F32, tag="ctx", bufs=1)
            for ec in range(2):
                psc = p_c.tile([1, 512], F32, tag="c")
                for si in range(NST):
                    chunk = enc_chunks[(b, si // 4)]
                    nc.tensor.matmul(
                        psc[:],
                        attnT[:, 2 * si:2 * si + 1],
                        chunk[:, si % 4, ec * 512:(ec + 1) * 512],
                        start=(si == 0), stop=(si == NST - 1),
                    )
                nc.vector.tensor_copy(crow[0:1, ec * 512:(ec + 1) * 512], psc[:])
            nc.sync.dma_start(ctx_out[b:b + 1, :], crow[:])
            # free natural chunks of batch b
            for sc in range(NSC):
                del enc_chunks[(b, sc)]

        # prologue
        emit_mask_row(0)
        emit_chunk_dma(*secs[0])
        emit_chunk_dma(*secs[1])

        for i, (b, sc) in enumerate(secs):
            if i + 2 < len(secs):
                emit_chunk_dma(*secs[i + 2])
            if sc == 0 and b + 1 < BL:
                emit_mask_row(b + 1)

            chunk = enc_chunks[(b, sc)]
            # transposes: encT tiles [128e, 512s] for this s-chunk
            encT = []
            for et in range(NET):
                ps = p_t.tile([128, 512], F32R, tag="t")
                for sb in range(4):
                    nc.tensor.transpose(
                        ps[:, sb * 128:(sb + 1) * 128],
                        chunk[:, sb, et * 128:(et + 1) * 128],
                        ident[:],
                    )
                et_t = encT_pool.tile([128, 512], F32R, tag="encT")
                if et % 2 == 0:
                    nc.vector.tensor_copy(et_t[:], ps[:])
                else:
                    nc.scalar.copy(et_t[:], ps[:])
                encT.append(et_t)

            # main matmul + tanh
            egyT = []
            for at in range(NAT):
                psm = p_m.tile([128, 512], F32, tag="m")
                for et in range(NET):
                    nc.tensor.matmul(
                        psm[:],
                        w_encT[et][:, at * 128:(at + 1) * 128],
                        encT[et][:],
                        start=(et == 0), stop=(et == NET - 1),
                    )
                eg = egyT_pool.tile([128, 512], F32R, tag="egyT")
                nc.scalar.activation(
                    eg[:], psm[:], Act.Tanh,
                    bias=dec_featT[:, at * BL + b:at * BL + b + 1],
                )
                egyT.append(eg)

            # scores for this s-chunk
            pss = p_s.tile([1, 512], F32, tag="s")
            for at in range(NAT):
                nc.tensor.matmul(
                    pss[:], w_eneT[:, 2 * at:2 * at + 1], egyT[at][:],
                    start=(at == 0), stop=(at == NAT - 1),
                )
            if sc == 0:
                scores_rows[b] = rows.tile([1, S], F32, tag="scores",
                                           name=f"scores{b}")
            nc.vector.tensor_tensor(
                scores_rows[b][0:1, sc * 512:(sc + 1) * 512],
                pss[:],
                mask_rows[b][0:1, sc * 512:(sc + 1) * 512],
                Alu.add,
            )

            # deferred per-batch tail work (previous batch), after this
            # section's PE stream so the softmax latency is hidden
            if sc == 0 and b > 0:
                emit_attnT_ctx(b - 1)

            if sc == NSC - 1:
                # softmax (no max subtraction: live scores are O(1), masked
                # lanes are -3.4e38 -> exp underflows to exactly 0)
                sm = small.tile([1, 1], F32, tag="sum")
                ar = rows.tile([1, S], F32R, tag="attn")
                nc.scalar.activation(ar[:], scores_rows[b][:], Act.Exp,
                                     accum_out=sm[:])
                rc = small.tile([1, 1], F32, tag="rec")
                nc.vector.reciprocal(rc[:], sm[:])
                nc.vector.tensor_scalar(ar[:], ar[:], rc[0:1, 0:1], None,
                                        op0=Alu.mult)
                attn_rows[b] = ar
                nc.sync.dma_start(attn_out[b:b + 1, :], ar[:])
                del scores_rows[b]

        # drain: last batch's tail
        emit_attnT_ctx(BL - 1)

    nc.finalize()
    return nc


_NC = None


def _get_program():
    global _NC
    if _NC is None:
        _NC = build_program()
    return _NC


def kernel(decoder_hidden, encoder_outputs, mask, W_enc, W_dec, W_energy):
    nc = _get_program()
    decoder_hidden = np.ascontiguousarray(np.asarray(decoder_hidden, dtype=np.float32))
    encoder_outputs = np.ascontiguousarray(np.asarray(encoder_outputs, dtype=np.float32))
    mask_u8 = np.ascontiguousarray(np.asarray(mask)).astype(np.uint8)
    W_enc = np.ascontiguousarray(np.asarray(W_enc, dtype=np.float32))
    W_dec = np.ascontiguousarray(np.asarray(W_dec, dtype=np.float32))
    W_energy = np.ascontiguousarray(np.asarray(W_energy, dtype=np.float32))

    in_maps = []
    for c in range(NCORES):
        sl = slice(c * BL, (c + 1) * BL)
        in_maps.append({
            "enc": encoder_outputs[sl],
            "dh": decoder_hidden[sl],
            "mask": mask_u8[sl],
            "w_enc": W_enc,
            "w_dec": W_dec,
            "w_energy": W_energy,
        })
    res = run_bass_kernel_spmd(nc, in_maps, list(range(NCORES)))
    context = np.concatenate([res.results[c]["ctx"] for c in range(NCORES)], axis=0)
    attn = np.concatenate([res.results[c]["attn"] for c in range(NCORES)], axis=0)
    return context.astype(np.float32), attn.astype(np.float32)


# revision 27
# speedup vs baseline: 217.5378x; 217.5378x over previous
"""Bahdanau attention TRN2 Bass kernel.

Full inputs in, full outputs out; batch-sharded across 8 NeuronCores
(8 batches per core), computed entirely on-device per core (no collectives).

Per-core per-batch dataflow (all fp32 bytes; PE ops run in float32r mode,
~12-bit-mantissa rounding, max rel err ~2.4e-4):
  1. DMA enc[b] (2048x1024) into SBUF natural chunks [128s, 4, 1024e].
  2. PE-transpose enc blocks -> encT tiles [128e, 512s]  (contraction over E
     needs E on partitions; fp32 DMA transpose is unsupported on TRN2).
  3. Main matmul: enc_featT[a,s] = W_encT.T @ encT, accumulated over 8 e-tiles.
  4. ACT tanh with per-partition bias dec_featT[:,b] straight out of PSUM.
  5. Scores: W_energyT.T @ energyT (M=1 matvec on PE), + mask*FLT_MIN via the
     PSUM->SBUF copy as a DVE tensor_tensor add.
  6. Softmax without max-subtraction (scores are O(1); masked lanes are
     -3.4e38 -> exp==0): ACT Exp with fused accum_out sum, DVE reciprocal,
     DVE tensor_scalar normalize.
  7. Context: attnT (PE-transposed attention row) as stationary, natural enc
     chunks as moving operand, accumulated over 16 s-tiles.
Batches are software-pipelined: batch b's attnT/context work is emitted after
batch b+1's first section of transposes/matmuls so the PE never stalls on the
softmax latency.
"""
import sys

sys.path.insert(0, "/opt/trn_rl_repo")

import numpy as np
from contextlib import ExitStack

import concourse.bass as bass
import concourse.tile as tile
from concourse import bacc, mybir
from concourse.bass_utils import run_bass_kernel_spmd
from concourse.masks import make_identity

dt = mybir.dt
F32 = dt.float32
F32R = dt.float32r
U8 = dt.uint8
Alu = mybir.AluOpType
Act = mybir.ActivationFunctionType

B, S, E, A = 64, 2048, 1024, 512
NCORES = 8
BL = B // NCORES            # 8 batches per core
NST = S // 128              # 16 s-tiles per batch
NSC = 4                     # s-chunks per batch (512 s values each)
NET = E // 128              # 8 e-tiles
NAT = A // 128              # 4 a-tiles
FMIN = float(np.finfo(np.float32).min)


def build_program(repeat=1):
    nc = bacc.Bacc(None, target_bir_lowering=False)

    enc_in = nc.declare_dram_parameter("enc", [BL, S, E], F32R, isOutput=False)
    dh_in = nc.declare_dram_parameter("dh", [BL, E], F32R, isOutput=False)
    mask_in = nc.declare_dram_parameter("mask", [BL, S], U8, isOutput=False)
    wenc_in = nc.declare_dram_parameter("w_enc", [A, E], F32R, isOutput=False)
    wdec_in = nc.declare_dram_parameter("w_dec", [A, E], F32R, isOutput=False)
    wene_in = nc.declare_dram_parameter("w_energy", [1, A], F32R, isOutput=False)
    ctx_out = nc.declare_dram_parameter("ctx", [BL, E], F32, isOutput=True)
    attn_out = nc.declare_dram_parameter("attn", [BL, S], F32R, isOutput=True)

    with tile.TileContext(nc) as tc, ExitStack() as ctx:
        persist = ctx.enter_context(tc.tile_pool(name="persist", bufs=1))
        p_t = ctx.enter_context(tc.tile_pool(name="p_t", bufs=4, space="PSUM"))
        p_m = ctx.enter_context(tc.tile_pool(name="p_m", bufs=2, space="PSUM"))
        p_s = ctx.enter_context(tc.tile_pool(name="p_s", bufs=1, space="PSUM"))
        p_c = ctx.enter_context(tc.tile_pool(name="p_c", bufs=1, space="PSUM"))

        # ---- steady-state pools (opened before startup so the startup pool
        # sits on top of the stack allocator and frees cleanly) ----
        enc_pool = ctx.enter_context(tc.tile_pool(name="encp", bufs=6))
        rows = ctx.enter_context(tc.tile_pool(name="rows", bufs=2))
        small = ctx.enter_context(tc.tile_pool(name="small", bufs=2))

        sums4 = {}
        secs = [(b, sc) for b in range(BL) for sc in range(NSC)]
        encT_pool = None  # opened after the startup pool releases
        egyT_pool = None
        enc_chunks = {}      # (b, sc) -> tile [128, 4, 1024]
        mask_rows = {}       # b -> [1, S] f32 tile
        exp_rows = {}        # b -> [1, S] f32r (unnormalized exp)
        recips = {}          # b -> [1, 1] f32 (1/sum)

        def emit_chunk_dma(b, sc):
            t = enc_pool.tile([128, NSC, E], F32R, tag="enc", name=f"enc{b}_{sc}")
            src = enc_in[b, sc * 512:(sc + 1) * 512, :].rearrange(
                "(t p) e -> p t e", p=128)
            nc.sync.dma_start(t[:], src)
            enc_chunks[(b, sc)] = t

        def emit_mask_row(b):
            mu = small.tile([1, S], U8, tag="mask_u8", name=f"mu{b}")
            nc.sync.dma_start(mu[:], mask_in[b:b + 1, :])
            mf = rows.tile([1, S], F32, tag="mask_f", name=f"mf{b}")
            nc.vector.tensor_scalar(mf[:], mu[:], FMIN, None, op0=Alu.mult)
            mask_rows[b] = mf

        # ---- startup: weights ----
        w_encT = []   # 8 tiles [128e, 512a] f32r
        dec_featT = persist.tile([128, NAT * BL], F32, tag="dfT")  # col = at*8+b
        w_eneT = persist.tile([128, 2 * NAT], F32R, tag="weT")  # cols 2*at (odd cols zero)
        ident = persist.tile([128, 128], F32R, tag="ident")

        with tc.tile_pool(name="startup", bufs=1) as st:
            ident32 = st.tile([128, 128], F32, tag="ident32")
            make_identity(nc, ident32[:])
            nc.vector.tensor_copy(ident[:], ident32[:])
            # DMA order mirrors PE's in-order startup needs: weights first
            # (W_encT transposes are the PE's first instructions), then the
            # first enc chunks + mask
            dh_sb = st.tile([BL, E], F32R, tag="dh")
            wene_sb = st.tile([1, A], F32R, tag="wene")
            wn_tiles = []
            wd_tiles = []
            for at in range(NAT):
                wn = st.tile([128, E], F32R, tag=f"wnat{at}", name=f"wen{at}")
                nc.sync.dma_start(wn[:], wenc_in[at * 128:(at + 1) * 128, :])
                wn_tiles.append(wn)
            nc.sync.dma_start(dh_sb[:], dh_in[:])
            nc.sync.dma_start(wene_sb[:], wene_in[:])
            for at in range(NAT):
                wd = st.tile([128, E], F32R, tag=f"wdnat{at}", name=f"wdn{at}")
                nc.sync.dma_start(wd[:], wdec_in[at * 128:(at + 1) * 128, :])
                wd_tiles.append(wd)
            emit_mask_row(0)
            emit_chunk_dma(*secs[0])
            emit_chunk_dma(*secs[1])

            # W_encT tiles (persist pool): stream natural a-tiles one at a
            # time, PE-transpose 128x128 blocks into each W_encT[et] column
            for et in range(NET):
                wt = persist.tile([128, A], F32R, tag=f"wencT{et}", name=f"wencT{et}")
                w_encT.append(wt)
            for at in range(NAT):
                for et in range(NET):
                    ps = p_t.tile([128, 128], F32R, tag="t", name=f"pswe{at}_{et}")
                    nc.tensor.transpose(ps[:], wn_tiles[at][:, et * 128:(et + 1) * 128],
                                        ident[:])
                    if et % 2 == 0:
                        nc.vector.tensor_copy(
                            w_encT[et][:, at * 128:(at + 1) * 128], ps[:])
                    else:
                        nc.scalar.copy(
                            w_encT[et][:, at * 128:(at + 1) * 128], ps[:])

            # dhT tiles [128d, 8b]
            dhT = []
            for et in range(NET):
                psd = p_t.tile([128, BL], F32R, tag="t", name=f"psdh{et}")
                nc.tensor.transpose(psd[:], dh_sb[:, et * 128:(et + 1) * 128],
                                    ident[0:BL, 0:BL])
                dht = st.tile([128, BL], F32R, tag=f"dhT{et}", name=f"dhT{et}")
                nc.vector.tensor_copy(dht[:], psd[:])
                dhT.append(dht)

            # dec_feat in [b, a] orientation with a streamed W_decT tile,
            # then transpose to dec_featT [a, b] for the ACT bias
            psdf = p_m.tile([BL, A], F32, tag="m", name="psdf")
            wd_nats = wd_tiles
            for et in range(NET):
                pswd = p_t.tile([128, A], F32R, tag="t", name=f"pswd{et}")
                for at in range(NAT):
                    nc.tensor.transpose(
                        pswd[:, at * 128:(at + 1) * 128],
                        wd_nats[at][:, et * 128:(et + 1) * 128], ident[:])
                wdt = st.tile([128, A], F32R, tag="wdecT", name=f"wdT{et}")
                nc.vector.tensor_copy(wdt[:], pswd[:])
                nc.tensor.matmul(psdf[:], dhT[et][:], wdt[:],
                                 start=(et == 0), stop=(et == NET - 1))
            df_nat = st.tile([BL, A], F32, tag="dfnat", name="dfnat")
            nc.vector.tensor_copy(df_nat[:], psdf[:])
            for at in range(NAT):
                psb = p_m.tile([128, BL], F32, tag="m", name=f"psdf{at}")
                nc.tensor.transpose(psb[:], df_nat[:, at * 128:(at + 1) * 128],
                                    ident32[0:BL, 0:BL])
                nc.scalar.copy(dec_featT[:, at * BL:(at + 1) * BL], psb[:])

            # W_energyT: fp32r moving operand needs even innermost count, so
            # transpose against ident[0:1, 0:2] = [1, 0] -> real col + zero col
            pse = p_t.tile([128, 2 * NAT], F32R, tag="t", name="pswe")
            for at in range(NAT):
                nc.tensor.transpose(
                    pse[:, 2 * at:2 * at + 2],
                    wene_sb[0:1, at * 128:(at + 1) * 128],
                    ident[0:1, 0:2],
                )
            nc.vector.tensor_copy(w_eneT[:], pse[:])

        # startup pool released; open the remaining steady pools on the
        # freed stack space
        encT_pool = ctx.enter_context(tc.tile_pool(name="encTp", bufs=10))
        egyT_pool = ctx.enter_context(tc.tile_pool(name="egyTp", bufs=6))

        def emit_attnT_ctx(b):
            # unnormalized exp row -> attnT via PE transposes; the 1/sum
            # normalization is folded into the context PSUM->SBUF copies
            arow = exp_rows[b]
            psa = p_t.tile([128, 2 * NST], F32R, tag="t")
            for si in range(NST):
                nc.tensor.transpose(
                    psa[:, 2 * si:2 * si + 2],
                    arow[0:1, si * 128:(si + 1) * 128],
                    ident[0:1, 0:2],
                )
            attnT = small.tile([128, 2 * NST], F32R, tag="attnT")
            nc.vector.tensor_copy(attnT[:], psa[:])
            # context: ctx[1, e] = sum_s attn[s] * enc[s, e]
            crow = rows.tile([1, E], F32, tag="ctx", bufs=1)
            for ec in range(2):
                psc = p_c.tile([1, 512], F32, tag="c")
                for si in range(NST):
                    chunk = enc_chunks[(b, si // 4)]
                    nc.tensor.matmul(
                        psc[:],
                        attnT[:, 2 * si:2 * si + 1],
                        chunk[:, si % 4, ec * 512:(ec + 1) * 512],
                        start=(si == 0), stop=(si == NST - 1),
                    )
                nc.vector.tensor_scalar(crow[0:1, ec * 512:(ec + 1) * 512],
                                        psc[:], recips[b][0:1, 0:1], None,
                                        op0=Alu.mult)
            nc.sync.dma_start(ctx_out[b:b + 1, :], crow[:])
            # normalized attention row out (fully off the critical path)
            ar = rows.tile([1, S], F32R, tag="attn", bufs=1, name=f"at{b}")
            nc.vector.tensor_scalar(ar[:], exp_rows[b][:], recips[b][0:1, 0:1],
                                    None, op0=Alu.mult)
            nc.sync.dma_start(attn_out[b:b + 1, :], ar[:])
            # free natural chunks of batch b
            for sc in range(NSC):
                del enc_chunks[(b, sc)]

        for rep in range(repeat):
          if rep > 0:
            emit_mask_row(0)
            emit_chunk_dma(*secs[0])
            emit_chunk_dma(*secs[1])
          for i, (b, sc) in enumerate(secs):
            if i + 2 < len(secs):
                emit_chunk_dma(*secs[i + 2])
            if sc == 0 and b + 1 < BL:
                emit_mask_row(b + 1)

            chunk = enc_chunks[(b, sc)]
            # transposes: encT tiles [128e, 512s] for this s-chunk
            encT = []
            for et in range(NET):
                ps = p_t.tile([128, 512], F32R, tag="t")
                for sb in range(4):
                    nc.tensor.transpose(
                        ps[:, sb * 128:(sb + 1) * 128],
                        chunk[:, sb, et * 128:(et + 1) * 128],
                        ident[:],
                    )
                et_t = encT_pool.tile([128, 512], F32R, tag="encT")
                if et % 4 == 0:
                    nc.vector.tensor_copy(et_t[:], ps[:])
                else:
                    nc.scalar.copy(et_t[:], ps[:])
                encT.append(et_t)

            # main matmul + tanh
            egyT = []
            for at in range(NAT):
                psm = p_m.tile([128, 512], F32, tag="m")
                for et in range(NET):
                    nc.tensor.matmul(
                        psm[:],
                        w_encT[et][:, at * 128:(at + 1) * 128],
                        encT[et][:],
                        start=(et == 0), stop=(et == NET - 1),
                    )
                eg = egyT_pool.tile([128, 512], F32R, tag="egyT")
                nc.scalar.activation(
                    eg[:], psm[:], Act.Tanh,
                    bias=dec_featT[:, at * BL + b:at * BL + b + 1],
                )
                egyT.append(eg)

            # scores for this s-chunk
            pss = p_s.tile([1, 512], F32, tag="s")
            for at in range(NAT):
                nc.tensor.matmul(
                    pss[:], w_eneT[:, 2 * at:2 * at + 1], egyT[at][:],
                    start=(at == 0), stop=(at == NAT - 1),
                )
            if sc == 0:
                exp_rows[b] = rows.tile([1, S], F32R, tag="exp",
                                        name=f"exp{b}")
                sums4[b] = small.tile([1, NSC], F32, tag="sums4",
                                      name=f"sums{b}")
            srow = small.tile([1, 512], F32, tag="scoresc", name=f"sr{b}_{sc}")
            nc.vector.tensor_tensor(
                srow[:], pss[:],
                mask_rows[b][0:1, sc * 512:(sc + 1) * 512], Alu.add)
            nc.scalar.activation(
                exp_rows[b][0:1, sc * 512:(sc + 1) * 512], srow[:],
                Act.Exp, accum_out=sums4[b][0:1, sc:sc + 1])
            if sc == NSC - 1:
                sm = small.tile([1, 1], F32, tag="sum", name=f"sm{b}")
                nc.vector.reduce_sum(sm[:], sums4[b][:],
                                     axis=mybir.AxisListType.X)
                rc = small.tile([1, 1], F32, tag="rec", name=f"rc{b}")
                nc.vector.reciprocal(rc[:], sm[:])
                recips[b] = rc
                del sums4[b]

            # deferred per-batch tail work (previous batch), after this
            # section's PE stream so the softmax latency is hidden
            if sc == 0 and b > 0:
                emit_attnT_ctx(b - 1)

          # drain: last batch's tail
          emit_attnT_ctx(BL - 1)

    nc.finalize()
    return nc


_NC = None


def _get_program():
    global _NC
    if _NC is None:
        _NC = build_program()
    return _NC


def kernel(decoder_hidden, encoder_outputs, mask, W_enc, W_dec, W_energy):
    nc = _get_program()
    decoder_hidden = np.ascontiguousarray(np.asarray(decoder_hidden, dtype=np.float32))
    encoder_outputs = np.ascontiguousarray(np.asarray(encoder_outputs, dtype=np.float32))
    mask_u8 = np.ascontiguousarray(np.asarray(mask)).astype(np.uint8)
    W_enc = np.ascontiguousarray(np.asarray(W_enc, dtype=np.float32))
    W_dec = np.ascontiguousarray(np.asarray(W_dec, dtype=np.float32))
    W_energy = np.ascontiguousarray(np.asarray(W_energy, dtype=np.float32))

    in_maps = []
    for c in range(NCORES):
        sl = slice(c * BL, (c + 1) * BL)
        in_maps.append({
            "enc": encoder_outputs[sl],
            "dh": decoder_hidden[sl],
            "mask": mask_u8[sl],
            "w_enc": W_enc,
            "w_dec": W_dec,
            "w_energy": W_energy,
        })
    res = run_bass_kernel_spmd(nc, in_maps, list(range(NCORES)))
    context = np.concatenate([res.results[c]["ctx"] for c in range(NCORES)], axis=0)
    attn = np.concatenate([res.results[c]["attn"] for c in range(NCORES)], axis=0)
    return context.astype(np.float32), attn.astype(np.float32)


# revision 36
# speedup vs baseline: 246.4449x; 1.1329x over previous
"""Bahdanau attention TRN2 Bass kernel.

Full inputs in, full outputs out; batch-sharded across 8 NeuronCores
(8 batches per core), computed entirely on-device per core (no collectives).

Per-core per-batch dataflow (all fp32 bytes; PE ops run in float32r mode,
~12-bit-mantissa rounding, max rel err ~2.4e-4):
  1. DMA enc[b] (2048x1024) into SBUF natural chunks [128s, 4, 1024e].
  2. PE-transpose enc blocks -> encT tiles [128e, 512s]  (contraction over E
     needs E on partitions; fp32 DMA transpose is unsupported on TRN2).
  3. Main matmul: enc_featT[a,s] = W_encT.T @ encT, accumulated over 8 e-tiles.
  4. ACT tanh with per-partition bias dec_featT[:,b] straight out of PSUM.
  5. Scores: W_energyT.T @ energyT (M=1 matvec on PE), + mask*FLT_MIN via the
     PSUM->SBUF copy as a DVE tensor_tensor add.
  6. Softmax without max-subtraction (scores are O(1); masked lanes are
     -3.4e38 -> exp==0): ACT Exp with fused accum_out sum, DVE reciprocal,
     DVE tensor_scalar normalize.
  7. Context: attnT (PE-transposed attention row) as stationary, natural enc
     chunks as moving operand, accumulated over 16 s-tiles.
Batches are software-pipelined: batch b's attnT/context work is emitted after
batch b+1's first section of transposes/matmuls so the PE never stalls on the
softmax latency.
"""
import sys

sys.path.insert(0, "/opt/trn_rl_repo")

import numpy as np
from contextlib import ExitStack

import concourse.bass as bass
import concourse.tile as tile
from concourse import bacc, mybir
from concourse.bass_utils import run_bass_kernel_spmd
from concourse.masks import make_identity

dt = mybir.dt
F32 = dt.float32
F32R = dt.float32r
U8 = dt.uint8
Alu = mybir.AluOpType
Act = mybir.ActivationFunctionType

B, S, E, A = 64, 2048, 1024, 512
NCORES = 8
BL = B // NCORES            # 8 batches per core
NST = S // 128              # 16 s-tiles per batch
NSC = 4                     # s-chunks per batch (512 s values each)
NET = E // 128              # 8 e-tiles
NAT = A // 128              # 4 a-tiles
FMIN = float(np.finfo(np.float32).min)


def build_program(repeat=1):
    nc = bacc.Bacc(None, target_bir_lowering=False)

    enc_in = nc.declare_dram_parameter("enc", [BL, S, E], F32R, isOutput=False)
    dh_in = nc.declare_dram_parameter("dh", [BL, E], F32R, isOutput=False)
    mask_in = nc.declare_dram_parameter("mask", [BL, S], U8, isOutput=False)
    wenc_in = nc.declare_dram_parameter("w_enc", [A, E], F32R, isOutput=False)
    wdec_in = nc.declare_dram_parameter("w_dec", [A, E], F32R, isOutput=False)
    wene_in = nc.declare_dram_parameter("w_energy", [1, A], F32R, isOutput=False)
    ctx_out = nc.declare_dram_parameter("ctx", [BL, E], F32, isOutput=True)
    attn_out = nc.declare_dram_parameter("attn", [BL, S], F32R, isOutput=True)

    with tile.TileContext(nc) as tc, ExitStack() as ctx:
        persist = ctx.enter_context(tc.tile_pool(name="persist", bufs=1))
        p_t = ctx.enter_context(tc.tile_pool(name="p_t", bufs=4, space="PSUM"))
        p_m = ctx.enter_context(tc.tile_pool(name="p_m", bufs=2, space="PSUM"))
        p_s = ctx.enter_context(tc.tile_pool(name="p_s", bufs=1, space="PSUM"))
        p_c = ctx.enter_context(tc.tile_pool(name="p_c", bufs=1, space="PSUM"))

        # ---- steady-state pools (opened before startup so the startup pool
        # sits on top of the stack allocator and frees cleanly) ----
        enc_pool = ctx.enter_context(tc.tile_pool(name="encp", bufs=6))
        rows = ctx.enter_context(tc.tile_pool(name="rows", bufs=2))
        small = ctx.enter_context(tc.tile_pool(name="small", bufs=2))

        sums4 = {}
        secs = [(b, sc) for b in range(BL) for sc in range(NSC)]
        encT_pool = None  # opened after the startup pool releases
        egyT_pool = None
        enc_chunks = {}      # (b, sc) -> tile [128, 4, 1024]
        mask_rows = {}       # b -> [1, S] f32 tile
        exp_rows = {}        # b -> [1, S] f32r (unnormalized exp)
        recips = {}          # b -> [1, 1] f32 (1/sum)

        def emit_chunk_dma(b, sc):
            t = enc_pool.tile([128, NSC, E], F32R, tag="enc", name=f"enc{b}_{sc}")
            src = enc_in[b, sc * 512:(sc + 1) * 512, :].rearrange(
                "(t p) e -> p t e", p=128)
            nc.sync.dma_start(t[:], src)
            enc_chunks[(b, sc)] = t

        def emit_mask_row(b):
            mu = small.tile([1, S], U8, tag="mask_u8", name=f"mu{b}")
            nc.sync.dma_start(mu[:], mask_in[b:b + 1, :])
            mf = rows.tile([1, S], F32, tag="mask_f", name=f"mf{b}")
            nc.vector.tensor_scalar(mf[:], mu[:], FMIN, None, op0=Alu.mult)
            mask_rows[b] = mf

        # ---- startup: weights ----
        w_encT = []   # 8 tiles [128e, 512a] f32r
        dec_featT = persist.tile([128, NAT * BL], F32, tag="dfT")  # col = at*8+b
        w_eneT = persist.tile([128, 2 * NAT], F32R, tag="weT")  # cols 2*at (odd cols zero)
        ident = persist.tile([128, 128], F32R, tag="ident")

        with tc.tile_pool(name="startup", bufs=1) as st:
            ident32 = st.tile([128, 128], F32, tag="ident32")
            make_identity(nc, ident32[:])
            nc.vector.tensor_copy(ident[:], ident32[:])
            # DMA order mirrors PE's in-order startup needs: weights first
            # (W_encT transposes are the PE's first instructions), then the
            # first enc chunks + mask
            dh_sb = st.tile([BL, E], F32R, tag="dh")
            wene_sb = st.tile([1, A], F32R, tag="wene")
            wn_tiles = []
            wd_tiles = []
            for at in range(NAT):
                wn = st.tile([128, E], F32R, tag=f"wnat{at}", name=f"wen{at}")
                nc.sync.dma_start(wn[:], wenc_in[at * 128:(at + 1) * 128, :])
                wn_tiles.append(wn)
            nc.sync.dma_start(dh_sb[:], dh_in[:])
            nc.sync.dma_start(wene_sb[:], wene_in[:])
            for at in range(NAT):
                wd = st.tile([128, E], F32R, tag=f"wdnat{at}", name=f"wdn{at}")
                nc.sync.dma_start(wd[:], wdec_in[at * 128:(at + 1) * 128, :])
                wd_tiles.append(wd)
            emit_mask_row(0)
            emit_chunk_dma(*secs[0])
            emit_chunk_dma(*secs[1])

            # W_encT tiles (persist pool): stream natural a-tiles one at a
            # time, PE-transpose 128x128 blocks into each W_encT[et] column
            for et in range(NET):
                wt = persist.tile([128, A], F32R, tag=f"wencT{et}", name=f"wencT{et}")
                w_encT.append(wt)
            for at in range(NAT):
                for et in range(NET):
                    ps = p_t.tile([128, 128], F32R, tag="t", name=f"pswe{at}_{et}")
                    nc.tensor.transpose(ps[:], wn_tiles[at][:, et * 128:(et + 1) * 128],
                                        ident[:])
                    if et % 2 == 0:
                        nc.vector.tensor_copy(
                            w_encT[et][:, at * 128:(at + 1) * 128], ps[:])
                    else:
                        nc.scalar.copy(
                            w_encT[et][:, at * 128:(at + 1) * 128], ps[:])

            # dhT tiles [128d, 8b]
            dhT = []
            for et in range(NET):
                psd = p_t.tile([128, BL], F32R, tag="t", name=f"psdh{et}")
                nc.tensor.transpose(psd[:], dh_sb[:, et * 128:(et + 1) * 128],
                                    ident[0:BL, 0:BL])
                dht = st.tile([128, BL], F32R, tag=f"dhT{et}", name=f"dhT{et}")
                nc.vector.tensor_copy(dht[:], psd[:])
                dhT.append(dht)

            # dec_feat in [b, a] orientation with a streamed W_decT tile,
            # then transpose to dec_featT [a, b] for the ACT bias
            psdf = p_m.tile([BL, A], F32, tag="m", name="psdf")
            wd_nats = wd_tiles
            for et in range(NET):
                pswd = p_t.tile([128, A], F32R, tag="t", name=f"pswd{et}")
                for at in range(NAT):
                    nc.tensor.transpose(
                        pswd[:, at * 128:(at + 1) * 128],
                        wd_nats[at][:, et * 128:(et + 1) * 128], ident[:])
                wdt = st.tile([128, A], F32R, tag="wdecT", name=f"wdT{et}")
                nc.vector.tensor_copy(wdt[:], pswd[:])
                nc.tensor.matmul(psdf[:], dhT[et][:], wdt[:],
                                 start=(et == 0), stop=(et == NET - 1))
            df_nat = st.tile([BL, A], F32, tag="dfnat", name="dfnat")
            nc.vector.tensor_copy(df_nat[:], psdf[:])
            for at in range(NAT):
                psb = p_m.tile([128, BL], F32, tag="m", name=f"psdf{at}")
                nc.tensor.transpose(psb[:], df_nat[:, at * 128:(at + 1) * 128],
                                    ident32[0:BL, 0:BL])
                nc.scalar.copy(dec_featT[:, at * BL:(at + 1) * BL], psb[:])

            # W_energyT: fp32r moving operand needs even innermost count, so
            # transpose against ident[0:1, 0:2] = [1, 0] -> real col + zero col
            pse = p_t.tile([128, 2 * NAT], F32R, tag="t", name="pswe")
            for at in range(NAT):
                nc.tensor.transpose(
                    pse[:, 2 * at:2 * at + 2],
                    wene_sb[0:1, at * 128:(at + 1) * 128],
                    ident[0:1, 0:2],
                )
            nc.vector.tensor_copy(w_eneT[:], pse[:])

        # startup pool released; open the remaining steady pools on the
        # freed stack space
        encT_pool = ctx.enter_context(tc.tile_pool(name="encTp", bufs=10))
        egyT_pool = ctx.enter_context(tc.tile_pool(name="egyTp", bufs=6))

        def emit_attnT_ctx(b):
            # unnormalized exp row -> attnT via PE transposes; the 1/sum
            # normalization is folded into the context PSUM->SBUF copies
            arow = exp_rows[b]
            psa = p_t.tile([128, 2 * NST], F32R, tag="t")
            for si in range(NST):
                nc.tensor.transpose(
                    psa[:, 2 * si:2 * si + 2],
                    arow[0:1, si * 128:(si + 1) * 128],
                    ident[0:1, 0:2],
                )
            attnT = small.tile([128, 2 * NST], F32R, tag="attnT")
            nc.vector.tensor_copy(attnT[:], psa[:])
            # context: ctx[1, e] = sum_s attn[s] * enc[s, e]
            crow = rows.tile([1, E], F32, tag="ctx", bufs=1)
            for ec in range(2):
                psc = p_c.tile([1, 512], F32, tag="c")
                for si in range(NST):
                    chunk = enc_chunks[(b, si // 4)]
                    nc.tensor.matmul(
                        psc[:],
                        attnT[:, 2 * si:2 * si + 1],
                        chunk[:, si % 4, ec * 512:(ec + 1) * 512],
                        start=(si == 0), stop=(si == NST - 1),
                    )
                nc.vector.tensor_scalar(crow[0:1, ec * 512:(ec + 1) * 512],
                                        psc[:], recips[b][0:1, 0:1], None,
                                        op0=Alu.mult)
            nc.sync.dma_start(ctx_out[b:b + 1, :], crow[:])
            # normalized attention row out (fully off the critical path)
            ar = rows.tile([1, S], F32R, tag="attn", bufs=1, name=f"at{b}")
            nc.vector.tensor_scalar(ar[:], exp_rows[b][:], recips[b][0:1, 0:1],
                                    None, op0=Alu.mult)
            nc.sync.dma_start(attn_out[b:b + 1, :], ar[:])
            # free natural chunks of batch b
            for sc in range(NSC):
                del enc_chunks[(b, sc)]

        for rep in range(repeat):
          if rep > 0:
            emit_mask_row(0)
            emit_chunk_dma(*secs[0])
            emit_chunk_dma(*secs[1])
          for i, (b, sc) in enumerate(secs):
            if i + 2 < len(secs):
                emit_chunk_dma(*secs[i + 2])
            if sc == 0 and b + 1 < BL:
                emit_mask_row(b + 1)

            chunk = enc_chunks[(b, sc)]
            # transposes: encT tiles [128e, 512s] for this s-chunk
            encT = []
            for et in range(NET):
                ps = p_t.tile([128, 512], F32R, tag="t")
                for sb in range(4):
                    nc.tensor.transpose(
                        ps[:, sb * 128:(sb + 1) * 128],
                        chunk[:, sb, et * 128:(et + 1) * 128],
                        ident[:],
                    )
                et_t = encT_pool.tile([128, 512], F32R, tag="encT")
                if et % 4 == 0:
                    nc.vector.tensor_copy(et_t[:], ps[:])
                else:
                    nc.scalar.copy(et_t[:], ps[:])
                encT.append(et_t)

            # main matmul + tanh
            egyT = []
            for at in range(NAT):
                psm = p_m.tile([128, 512], F32, tag="m")
                for et in range(NET):
                    nc.tensor.matmul(
                        psm[:],
                        w_encT[et][:, at * 128:(at + 1) * 128],
                        encT[et][:],
                        start=(et == 0), stop=(et == NET - 1),
                    )
                eg = egyT_pool.tile([128, 512], F32R, tag="egyT")
                nc.scalar.activation(
                    eg[:], psm[:], Act.Tanh,
                    bias=dec_featT[:, at * BL + b:at * BL + b + 1],
                )
                egyT.append(eg)

            # scores for this s-chunk
            pss = p_s.tile([1, 512], F32, tag="s")
            for at in range(NAT):
                nc.tensor.matmul(
                    pss[:], w_eneT[:, 2 * at:2 * at + 1], egyT[at][:],
                    start=(at == 0), stop=(at == NAT - 1),
                )
            if sc == 0:
                exp_rows[b] = rows.tile([1, S], F32R, tag="exp",
                                        name=f"exp{b}")
                sums4[b] = small.tile([1, NSC], F32, tag="sums4",
                                      name=f"sums{b}")
            srow = small.tile([1, 512], F32, tag="scoresc", name=f"sr{b}_{sc}")
            nc.vector.tensor_tensor(
                srow[:], pss[:],
                mask_rows[b][0:1, sc * 512:(sc + 1) * 512], Alu.add)
            nc.scalar.activation(
                exp_rows[b][0:1, sc * 512:(sc + 1) * 512], srow[:],
                Act.Exp, accum_out=sums4[b][0:1, sc:sc + 1])
            if sc == NSC - 1:
                sm = small.tile([1, 1], F32, tag="sum", name=f"sm{b}")
                nc.vector.reduce_sum(sm[:], sums4[b][:],
                                     axis=mybir.AxisListType.X)
                rc = small.tile([1, 1], F32, tag="rec", name=f"rc{b}")
                nc.vector.reciprocal(rc[:], sm[:])
                recips[b] = rc
                del sums4[b]

            # deferred per-batch tail work (previous batch), after this
            # section's PE stream so the softmax latency is hidden
            if sc == 0 and b > 0:
                emit_attnT_ctx(b - 1)

          # drain: last batch's tail
          emit_attnT_ctx(BL - 1)

    nc.finalize()
    return nc


_NC = None


def _get_program():
    global _NC
    if _NC is None:
        _NC = build_program()
    return _NC


def kernel(decoder_hidden, encoder_outputs, mask, W_enc, W_dec, W_energy):
    nc = _get_program()
    decoder_hidden = np.ascontiguousarray(np.asarray(decoder_hidden, dtype=np.float32))
    encoder_outputs = np.ascontiguousarray(np.asarray(encoder_outputs, dtype=np.float32))
    mask_u8 = np.ascontiguousarray(np.asarray(mask)).astype(np.uint8)
    W_enc = np.ascontiguousarray(np.asarray(W_enc, dtype=np.float32))
    W_dec = np.ascontiguousarray(np.asarray(W_dec, dtype=np.float32))
    W_energy = np.ascontiguousarray(np.asarray(W_energy, dtype=np.float32))

    in_maps = []
    for c in range(NCORES):
        sl = slice(c * BL, (c + 1) * BL)
        in_maps.append({
            "enc": encoder_outputs[sl],
            "dh": decoder_hidden[sl],
            "mask": mask_u8[sl],
            "w_enc": W_enc,
            "w_dec": W_dec,
            "w_energy": W_energy,
        })
    res = run_bass_kernel_spmd(nc, in_maps, list(range(NCORES)))
    context = np.concatenate([res.results[c]["ctx"] for c in range(NCORES)], axis=0)
    attn = np.concatenate([res.results[c]["attn"] for c in range(NCORES)], axis=0)
    return context.astype(np.float32), attn.astype(np.float32)
